# revision 11
# baseline (speedup 1.0000x reference)
"""Trainium2 Bass kernel for batched weighted scatter-add (AttentionCopy).

Computes out[b, o, v] = sum_i attn[b, o, i] * (ids[b, i] == v)
for ids [16, 512] int32 in [0, 50000), attn [16, 32, 512] f32,
out [16, 32, 50000] f32.

Strategy: pure data parallel over batch (2 batches per core on 8 cores).
The output is 99% zeros (<=512 of 50000 columns are non-zero per batch), so
instead of dense one-hot matmuls (PE-bound at ~50us), the kernel:

  1. Zero-fills the whole per-core output (12.8 MB) with large coalesced
     DMAs from an SBUF zeros tile -- this runs at the HBM write roofline
     and is the unavoidable cost of materializing the dense output.  The
     DMA partition dim must be 128 (the hardware sprays descriptors over
     engines by evenly dividing the partition count; 125 partitions would
     use only 5 of 16 engines).
  2. Resolves duplicate ids with the selection-matrix trick: C[j, s] =
     (ids_j == slot_s) built by DVE compares, then ST = C.T @ attnT on the
     PE (tiny matmuls) so every slot holds the full collision sum for its
     column.  Duplicate slots hold identical rows, making duplicate
     scatter writes benign (plain overwrite, no read-modify-write).
  3. Scatters the non-zero columns with indirect DMAs (one index per
     partition, each writing a contiguous 32-float row of the v-major
     octant tensor).

The output is split into 16 DRAM tensors (2 batches x 8 vocab octants:
seven of 6400 rows, one of 5200).  The tile framework tracks DRAM
write-write hazards per tensor, so each octant's scatter automatically
waits only for that octant's zero-fill DMA, and the 16 zero-fill/scatter
pipelines overlap.  Columns are bucketed into their octant's 128 index
slots on the host (pure index preprocessing); empty slots are padded
with a duplicate of a real column in the same octant (identical payload
-> benign) or, for an empty octant, with id -1 (all-zero payload written
to a row that has no real column).

The device output is v-major; the host unshard step reassembles octants
and transposes each batch to the required [32, 50000] row-major layout.
"""

import sys

sys.path.insert(0, "/opt/trn_rl_repo")

import numpy as np

NCORES = 8
B, O, I = 16, 32, 512
SIZE = 50000
BPC = B // NCORES  # batches per core
NCHUNK = I // 128  # 4 contraction chunks of 128
NOCT = 8  # vocab octants per batch
OSTEP = 6400  # octant row span (last octant: 5200)
OSIZES = [OSTEP] * 7 + [SIZE - 7 * OSTEP]
SLOTS = NOCT * 128  # 1024 column slots per batch (128 per octant)

_cache = {}


def _build():
    import concourse.bacc as bacc
    import concourse.bass as bass
    import concourse.mybir as mybir
    import concourse.tile as tile

    f32 = mybir.dt.float32
    bf16 = mybir.dt.bfloat16
    i32 = mybir.dt.int32
    Alu = mybir.AluOpType

    nc = bacc.Bacc("TRN2", target_bir_lowering=False, debug=False, num_devices=NCORES)

    # slot column ids (f32), replicated on all partitions:
    # idsb[p, b*1024 + q*128 + s] = id of slot (b, q, s)
    idsb_d = nc.dram_tensor("idsb", [128, BPC * SLOTS], f32, kind="ExternalInput").ap()
    # per-partition contraction ids: idspp[p, b*4 + c] = ids[b, c*128 + p]
    idspp_d = nc.dram_tensor("idspp", [128, BPC * NCHUNK], f32, kind="ExternalInput").ap()
    # octant-local scatter rows: idx[p, b*8 + q] = slot (b,q,p)'s local row
    idx_d = nc.dram_tensor("idx", [128, BPC * NOCT], i32, kind="ExternalInput").ap()
    # attn transposed: [BPC, I, O]
    attn_d = nc.dram_tensor("attn", [BPC, I, O], f32, kind="ExternalInput").ap()
    # v-major outputs, one tensor per (batch, octant):
    # out_b{b}q{q}[r, o] = out[b, o, q*6400 + r]
    out_d = [
        [
            nc.dram_tensor(f"out_b{b}q{q}", [OSIZES[q], O], f32, kind="ExternalOutput").ap()
            for q in range(NOCT)
        ]
        for b in range(BPC)
    ]

    with tile.TileContext(nc) as tc:
        with (
            tc.tile_pool(name="zeros", bufs=1) as zp,
            tc.tile_pool(name="inp", bufs=1) as inp,
            tc.tile_pool(name="work", bufs=1) as wp,
            tc.tile_pool(name="psst", bufs=4, space="PSUM") as psp,
        ):
            # --- inputs first: they gate all compute, so they must clear
            # the DMA queues before the 12.8 MB zero-fill enters
            idspp = inp.tile([128, BPC * NCHUNK], f32)
            nc.sync.dma_start(out=idspp[:], in_=idspp_d[:])
            idx = inp.tile([128, BPC * NOCT], i32)
            nc.sync.dma_start(out=idx[:], in_=idx_d[:])
            idsb = inp.tile([128, BPC * SLOTS], f32)
            for b in range(BPC):
                eng = (nc.sync, nc.scalar)[b % 2]
                eng.dma_start(
                    out=idsb[:, b * SLOTS : (b + 1) * SLOTS],
                    in_=idsb_d[:, b * SLOTS : (b + 1) * SLOTS],
                )
            at_f = inp.tile([128, BPC * NCHUNK * O], f32)
            for b in range(BPC):
                for c in range(NCHUNK):
                    eng = (nc.sync, nc.scalar)[c % 2]
                    eng.dma_start(
                        out=at_f[:, (b * NCHUNK + c) * O : (b * NCHUNK + c + 1) * O],
                        in_=attn_d[b][c * 128 : (c + 1) * 128, :],
                    )
            atb = inp.tile([128, BPC * NCHUNK * O], bf16)
            nc.vector.tensor_copy(out=atb[:], in_=at_f[:])

            # --- zeros tile + zero-fill DMAs (one per octant tensor, 128
            # partitions each so all 16 DMA engines are used)
            zt = zp.tile([128, OSTEP * O // 128], f32)
            nc.vector.memset(zt[:], 0)
            for b in range(BPC):
                for q in range(NOCT):
                    zc = OSIZES[q] * O // 128
                    eng = (nc.sync, nc.scalar)[(b * NOCT + q) % 2]
                    eng.dma_start(
                        out=out_d[b][q]
                        .rearrange("r o -> (r o)")
                        .rearrange("(p f) -> p f", f=zc),
                        in_=zt[:, 0:zc],
                    )

            for b in range(BPC):
                # --- C[j, s] = (ids_j == slot_s), bf16 0/1, per j-chunk
                cmat = wp.tile([128, NCHUNK * SLOTS], bf16, name=f"c{b}")
                for cj in range(NCHUNK):
                    nc.vector.tensor_scalar(
                        out=cmat[:, cj * SLOTS : (cj + 1) * SLOTS],
                        in0=idsb[:, b * SLOTS : (b + 1) * SLOTS],
                        scalar1=idspp[:, b * NCHUNK + cj : b * NCHUNK + cj + 1],
                        scalar2=None,
                        op0=Alu.is_equal,
                    )

                # --- per octant: ST = C.T @ attnT (collision sums), then scatter
                vals = wp.tile([128, NOCT * O], f32, name=f"v{b}")
                for q in range(NOCT):
                    pst = psp.tile([128, O], f32, tag="st")
                    for cj in range(NCHUNK):
                        nc.tensor.matmul(
                            out=pst[:],
                            lhsT=cmat[
                                :, cj * SLOTS + q * 128 : cj * SLOTS + (q + 1) * 128
                            ],
                            rhs=atb[:, (b * NCHUNK + cj) * O : (b * NCHUNK + cj + 1) * O],
                            start=(cj == 0),
                            stop=(cj == NCHUNK - 1),
                        )
                    nc.scalar.copy(out=vals[:, q * O : (q + 1) * O], in_=pst[:])

                    # indirect scatter: partition p writes vals[p, q*32:...]
                    # to row idx[p, b*8+q] of out_b{b}q{q}.  The DRAM WAW
                    # hazard on the octant tensor orders this after the
                    # octant's zero-fill DMA.
                    nc.gpsimd.indirect_dma_start(
                        out=out_d[b][q][:],
                        out_offset=bass.IndirectOffsetOnAxis(
                            ap=idx[:, b * NOCT + q : b * NOCT + q + 1], axis=0
                        ),
                        in_=vals[:, q * O : (q + 1) * O],
                        in_offset=None,
                    )

    nc.compile()
    return nc


def _in_maps(ids, attn):
    ids = np.asarray(ids, dtype=np.int64)
    in_maps = []
    for core in range(NCORES):
        idsc = ids[core * BPC : (core + 1) * BPC]  # [BPC, I]
        # per-partition contraction ids (f32 exact below 2**24)
        pp = (
            idsc.astype(np.float32)
            .reshape(BPC, NCHUNK, 128)
            .transpose(2, 0, 1)
            .reshape(128, BPC * NCHUNK)
        )
        # bucket columns into (octant, slot) with duplicate padding
        idsml = np.full((1, BPC * SLOTS), -1.0, dtype=np.float32)
        idxt = np.zeros((128, BPC * NOCT), dtype=np.int32)
        for b in range(BPC):
            oct_of = np.minimum(idsc[b] // OSTEP, NOCT - 1)
            for q in range(NOCT):
                cols = idsc[b][oct_of == q]  # this octant's column ids
                n = len(cols)
                assert n <= 128, f"octant overflow: {n} columns"
                if n:
                    slot = np.empty(128, dtype=np.int64)
                    slot[:n] = cols
                    slot[n:] = cols[0]  # duplicate pad: identical payload
                    idsml[0, (b * NOCT + q) * 128 : (b * NOCT + q + 1) * 128] = slot
                    idxt[:, b * NOCT + q] = slot - q * OSTEP
                # else: idsml stays -1 (all-zero payload), idx stays 0
        in_maps.append(
            {
                "idsb": np.ascontiguousarray(
                    np.broadcast_to(idsml, (128, BPC * SLOTS))
                ),
                "idspp": np.ascontiguousarray(pp),
                "idx": idxt,
                "attn": np.ascontiguousarray(
                    attn[core * BPC : (core + 1) * BPC].transpose(0, 2, 1)
                ),
            }
        )
    return in_maps


def kernel(ids, attn):
    from concourse.bass_utils import run_bass_kernel_spmd

    ids = np.ascontiguousarray(ids, dtype=np.int32)
    attn = np.ascontiguousarray(attn, dtype=np.float32)

    if "nc" not in _cache:
        _cache["nc"] = _build()
    nc = _cache["nc"]

    core_ids = list(range(NCORES))
    res = run_bass_kernel_spmd(nc, _in_maps(ids, attn), core_ids)
    # reassemble: per (core, batch) concat octants -> [50000, 32] -> transpose
    out = np.empty((B, O, SIZE), dtype=np.float32)
    for c in core_ids:
        for b in range(BPC):
            vmaj = np.concatenate(
                [res.results[c][f"out_b{b}q{q}"] for q in range(NOCT)], axis=0
            )
            out[c * BPC + b] = vmaj.T
    return out
